# revision 7
# baseline (speedup 1.0000x reference)
"""Trainium2 Bass kernel for the nn_Decoder LSTM problem (pipelined
mixed fp8/fp16 fixed point).

Teacher-forced LSTM decoder, T=8192 steps, D=1024; the candidate-cell gate
reads [h, c] instead of [h, x].

Strategy
--------
Scan-accelerated fixed point: time is sharded across 8 cores (1024 owned
steps + DELTA warm-up, zero cross-core traffic).  Per sweep, gate
pre-activations for the whole chunk are dense matmuls against the previous
iterate of (h, c) in [d, t] layout; the c-recurrence is solved exactly per
sweep with the DVE tensor_tensor_scan (fp32 state).  x-contributions are
precomputed once (phase 1) and streamed from a DRAM scratch.

Two precision phases: K_F8 fp8(e4m3) DoubleRow sweeps (2x MACs/cycle via
256-deep contraction; fp8 mirrors H8/C8 of the iterates, padded rows — odd
byte strides crash the device) approach the ~4.5e-2 fp8 floor; K_F16 f16
sweeps polish toward the ~7e-4 f16 fixed point.  The f16 weights load into
the SBUF freed when the fp8 pool closes.  Simulated rel-err 1.2e-2,
measured 0.9e-2 (gate 2e-2).

v5: the engines are in-order, so v4 lost ~25% of wall to PE stalls at
every m-tile boundary (Gauss-Seidel freshness made tile k+1's matmuls wait
on tile k's scan/activation tail).  v5 switches the matmul inputs to
Jacobi-across-tiles (previous-sweep values; simulated convergence is
identical — the c-scan boundary chaining carries the cross-tile
information), which drops the RAW edges; the per-tile stages are then
emitted interleaved (tile k's inner refinements between tile k+1/k+2's
gate groups) so every engine always has independent work queued:

    G0 G1 S00 R01 G2 S01 R02 S02 H0 S10 R11 S11 R12 S12 H1 S20 ... H2

(G=gates+ctilde matmuls/acts, R=inner ctilde refinement matmuls/acts,
S=b-mul+scan (the serial c-chain, in chain order on the DVE), H=h-tail.)
The ctilde h-part is cached (zA16) and re-added with an identity matmul —
no PSUM->SBUF f32 round trips.  All-but-last fp8 sweeps skip the f16 h
write (scan/h outputs go straight to the fp8 mirrors).  The boundary
-column deferral machinery is gone (scans emit all N columns).
"""

import sys
import numpy as np
import ml_dtypes

for _p in ("/opt/trn_rl_repo", "/root/.axon_site/_ro/trn_rl_repo"):
    if _p not in sys.path:
        sys.path.insert(0, _p)

import concourse.bass as bass
import concourse.bacc as bacc
import concourse.mybir as mybir
import concourse.tile as tile
from concourse.bass_utils import run_bass_kernel_spmd

D = 1024
T = 8192
KC = 8            # contraction chunks (1024/128)
DELTA = 64        # warm-up overlap steps
L = 1024 + DELTA  # chunk length per core
LP8 = L + 8       # fp8 mirror row extent (8-byte aligned strides)
N_CORES = 8
K_F8 = 6          # phase-A sweeps (incl. the zero-state shortcut sweep)
K_F16 = 2         # phase-B polish sweeps
N_INNER = 2       # inner c-refinements per sweep

F8 = mybir.dt.float8e4
F16 = mybir.dt.float16
F32 = mybir.dt.float32
AF = mybir.ActivationFunctionType
ALU = mybir.AluOpType
DR = mybir.MatmulPerfMode.DoubleRow

# time-tiles per sweep: (t0, N); 16-aligned offsets, N large enough that
# LDWEIGHTS stays hidden behind row streaming
M_TILES = [(0, 368), (368, 368), (736, 352)]
NMAX = 368


def _emit_pipeline(sweeps):
    """Cross-sweep software pipelining: each serial refinement chain (S/R/H
    of one tile) has the NEXT sweep's fat gate matmul block queued behind it
    on the in-order PE queue.  Viable now that PSUM gate/refinement tags are
    split (gate blocks rotation-wait only on earlier gate blocks, not on
    in-flight chain tails); bufs=2 gate-tag WAR edges stay acyclic because a
    tile's chain completes before the G-block two later that reuses it."""
    n = len(sweeps)

    def chain(s, k):
        sw = sweeps[s]
        sw.S(k, 0)
        for r in range(1, sw.ni + 1):
            sw.R(k, r)
            sw.S(k, r)
        sw.H(k)

    sweeps[0].G(0)
    sweeps[0].G(1)
    chain(0, 0)
    sweeps[0].G(2)
    for s in range(n):
        chain(s, 1)
        if s + 1 < n:
            sweeps[s + 1].G(0)
        chain(s, 2)
        if s + 1 < n:
            sweeps[s + 1].G(1)
            chain(s + 1, 0)
            sweeps[s + 1].G(2)


def build_nc(k_f8: int = K_F8, k_f16: int = K_F16, n_inner: int = N_INNER):
    nc = bacc.Bacc(None, target_bir_lowering=False, debug=False)

    # ---- I/O ----
    wh8_d = nc.declare_dram_parameter("wh8", [D, 4 * D], F8, isOutput=False)
    wc8_d = nc.declare_dram_parameter("wc8", [D, D], F8, isOutput=False)
    wh16_d = nc.declare_dram_parameter("wh16", [D, 4 * D], F16, isOutput=False)
    wc16_d = nc.declare_dram_parameter("wc16", [D, D], F16, isOutput=False)
    wx_t = nc.declare_dram_parameter("wx_t", [D, 3 * D], F16, isOutput=False)
    x_t = nc.declare_dram_parameter("x_t", [D, L], F16, isOutput=False)
    bias_f = nc.declare_dram_parameter("bias_f", [128, KC], F32, isOutput=False)
    bias_i = nc.declare_dram_parameter("bias_i", [128, KC], F32, isOutput=False)
    bias_o = nc.declare_dram_parameter("bias_o", [128, KC], F32, isOutput=False)
    bias_c = nc.declare_dram_parameter("bias_c", [128, KC], F32, isOutput=False)
    hb = nc.declare_dram_parameter("hb", [128, KC, 1], F16, isOutput=False)
    cb = nc.declare_dram_parameter("cb", [128, KC, 1], F16, isOutput=False)
    ident = nc.declare_dram_parameter("ident", [128, 128], F16, isOutput=False)
    h_out = nc.declare_dram_parameter("h_out", [128, KC, L + 1], F16, isOutput=True)

    # x-contribution of f,i,o gates, [gate*8+gd, 128, L], computed in phase 1
    pre_dram = nc.dram_tensor("pre_scratch", [24, 128, L], F16)

    with tile.TileContext(nc) as tc:
        with (
            tc.tile_pool(name="const", bufs=1) as constp,
            tc.tile_pool(name="psum", bufs=8, space="PSUM") as psum,
            tc.tile_pool(name="state", bufs=1) as spool,
        ):
            bf_sb = constp.tile([128, KC], F32, tag="bf")
            bi_sb = constp.tile([128, KC], F32, tag="bi")
            bo_sb = constp.tile([128, KC], F32, tag="bo")
            bc_sb = constp.tile([128, KC], F32, tag="bc")
            id_sb = constp.tile([128, 128], F16, tag="ident")
            nc.sync.dma_start(bf_sb[:, :], bias_f[:, :])
            nc.sync.dma_start(bi_sb[:, :], bias_i[:, :])
            nc.sync.dma_start(bo_sb[:, :], bias_o[:, :])
            nc.sync.dma_start(bc_sb[:, :], bias_c[:, :])
            nc.sync.dma_start(id_sb[:, :], ident[:, :])

            # h/c history, col j = value at local time j-1 (col 0 = boundary)
            Hf = spool.tile([128, KC, L + 1], F16, tag="H")
            Cf = spool.tile([128, KC, L + 1], F16, tag="C")
            nc.vector.memset(Hf[:, :, :], 0.0)
            nc.vector.memset(Cf[:, :, :], 0.0)
            nc.sync.dma_start(Hf[:, :, 0:1], hb[:, :, :])
            nc.sync.dma_start(Cf[:, :, 0:1], cb[:, :, :])

            # ---- phase 1: pre = Wx @ x (f16) ----
            with tc.tile_pool(name="ph1", bufs=3) as ph1:
                xT_sb = ph1.tile([128, KC, L], F16, tag="xT", bufs=1)
                nc.sync.dma_start(
                    xT_sb[:, :, :], x_t[:, :].rearrange("(c p) t -> p c t", p=128)
                )
                slabs = []
                for g in range(3):
                    wx_sb = ph1.tile([128, KC, D], F16, tag="wxslab")
                    nc.sync.dma_start(
                        wx_sb[:, :, :],
                        wx_t[:, g * D:(g + 1) * D].rearrange(
                            "(c p) m -> p c m", p=128
                        ),
                    )
                    slabs.append(wx_sb)
                for g in range(3):  # f, i, o
                    wx_sb = slabs[g]
                    for (t0, N) in M_TILES:
                        for gd in range(KC):
                            ps = psum.tile([128, N], F32, tag="ps")
                            for kc in range(KC):
                                nc.tensor.matmul(
                                    ps[:, :],
                                    wx_sb[:, kc, gd * 128:(gd + 1) * 128],
                                    xT_sb[:, kc, t0:t0 + N],
                                    start=(kc == 0),
                                    stop=(kc == KC - 1),
                                )
                            pre16 = ph1.tile([128, N], F16, tag="pre16")
                            nc.vector.tensor_copy(pre16[:, :], ps[:, :])
                            nc.sync.dma_start(
                                pre_dram[g * KC + gd, :, t0:t0 + N], pre16[:, :]
                            )

            with (
                tc.tile_pool(name="gates", bufs=2) as gpool,
                tc.tile_pool(name="work", bufs=2) as wk,
                tc.tile_pool(name="prestream", bufs=2) as prepool,
            ):

                def prefetch_pre(t0, N):
                    tiles = []
                    for base, tag in ((0, "pf"), (KC, "pi"), (2 * KC, "po")):
                        row = []
                        for gd in range(KC):
                            p = prepool.tile([128, N], F16, tag=tag, bufs=6)
                            nc.sync.dma_start(
                                p[:, :], pre_dram[base + gd, :, t0:t0 + N])
                            row.append(p)
                        tiles.append(row)
                    return tiles  # [pf_t, pi_t, po_t]

                # ============ PHASE A: fp8 sweeps ============
                with tc.tile_pool(name="phA", bufs=1) as pA:
                    wh8_sb = pA.tile([128, KC, 4 * D], F8, tag="wh8")
                    wc8_sb = pA.tile([128, KC, D], F8, tag="wc8")
                    H8 = pA.tile([128, KC, LP8], F8, tag="H8")
                    C8 = pA.tile([128, KC, LP8], F8, tag="C8")
                    nc.sync.dma_start(
                        wh8_sb[:, :, :],
                        wh8_d[:, :].rearrange("(c p) m -> p c m", p=128),
                    )
                    nc.sync.dma_start(
                        wc8_sb[:, :, :],
                        wc8_d[:, :].rearrange("(c p) m -> p c m", p=128),
                    )
                    nc.vector.memset(H8[:, :, :], 0.0)
                    nc.vector.memset(C8[:, :, :], 0.0)
                    nc.vector.tensor_copy(H8[:, :, 0:1], Hf[:, :, 0:1])
                    nc.vector.tensor_copy(C8[:, :, 0:1], Cf[:, :, 0:1])

                    def dr_group(ps, w_sb, src8, col0, t0, N, start, stop):
                        for j in range(KC // 2):
                            nc.tensor.matmul(
                                ps[:, :],
                                w_sb[:, 2 * j:2 * j + 2, col0:col0 + 128],
                                src8[:, 2 * j:2 * j + 2, t0:t0 + N],
                                start=start and (j == 0),
                                stop=stop and (j == KC // 2 - 1),
                                perf_mode=DR,
                            )

                    class _SweepObj:
                        def __init__(self, G, R, S, H, ni):
                            self.G, self.R, self.S, self.H, self.ni = G, R, S, H, ni

                    def make_sweep_f8(first, light, ni):
                        st = {}

                        def stage_G(k):
                            t0, N = M_TILES[k]
                            s = st[k] = {}
                            s["f"] = gpool.tile([128, KC, NMAX], F16, tag="f", name="fga")
                            s["i"] = gpool.tile([128, KC, NMAX], F16, tag="i", name="iga")
                            s["o"] = gpool.tile([128, KC, NMAX], F16, tag="o", name="oga")
                            s["ct"] = gpool.tile([128, KC, NMAX], F16, tag="ct", name="ctga")
                            s["zA"] = gpool.tile([128, KC, NMAX], F16,
                                                 tag="zA16", bufs=3, name="zA16")
                            pf_t, pi_t, po_t = prefetch_pre(t0, N)
                            s["pre"] = (pf_t, pi_t, po_t)
                            # ctilde r=0
                            for ch in range(KC):
                                if first:
                                    nc.scalar.activation(
                                        s["ct"][:, ch, :N], pf_t[ch][:, :],
                                        AF.Tanh, bias=bc_sb[:, ch:ch + 1],
                                        scale=0.0,
                                    )
                                    continue
                                psA = psum.tile([128, N], F32, tag="ps")
                                dr_group(psA, wh8_sb, H8, 3 * D + ch * 128,
                                         t0, N, True, True)
                                nc.vector.tensor_copy(s["zA"][:, ch, :N], psA[:, :])
                                ps0 = psum.tile([128, N], F32, tag="ps")
                                dr_group(ps0, wc8_sb, C8, ch * 128, t0, N,
                                         True, False)
                                nc.tensor.matmul(
                                    ps0[:, :], id_sb[:, :], s["zA"][:, ch, :N],
                                    start=False, stop=True,
                                )
                                nc.scalar.activation(
                                    s["ct"][:, ch, :N], ps0[:, :], AF.Tanh,
                                    bias=bc_sb[:, ch:ch + 1],
                                )
                            # f, i, o gates
                            for garr, pre_tiles, bias_sb, col0 in (
                                (s["f"], pf_t, bf_sb, 0),
                                (s["i"], pi_t, bi_sb, D),
                                (s["o"], po_t, bo_sb, 2 * D),
                            ):
                                for ch in range(KC):
                                    if first:
                                        nc.scalar.activation(
                                            garr[:, ch, :N], pre_tiles[ch][:, :],
                                            AF.Sigmoid, bias=bias_sb[:, ch:ch + 1],
                                        )
                                        continue
                                    ps = psum.tile([128, N], F32, tag="ps")
                                    dr_group(ps, wh8_sb, H8, col0 + ch * 128,
                                             t0, N, True, False)
                                    nc.tensor.matmul(
                                        ps[:, :], id_sb[:, :], pre_tiles[ch][:, :],
                                        start=False, stop=True,
                                    )
                                    nc.scalar.activation(
                                        garr[:, ch, :N], ps[:, :], AF.Sigmoid,
                                        bias=bias_sb[:, ch:ch + 1],
                                    )

                        def stage_R(k, r):
                            t0, N = M_TILES[k]
                            s = st[k]
                            cts = []
                            for ch in range(KC):
                                ps2 = psum.tile([128, N], F32, tag="ps")
                                if first:
                                    dr_group(ps2, wc8_sb, C8, ch * 128, t0, N,
                                             True, True)
                                else:
                                    dr_group(ps2, wc8_sb, C8, ch * 128, t0, N,
                                             True, False)
                                    nc.tensor.matmul(
                                        ps2[:, :], id_sb[:, :], s["zA"][:, ch, :N],
                                        start=False, stop=True,
                                    )
                                ct2 = wk.tile([128, N], F16, tag="ct2", bufs=4)
                                nc.scalar.activation(
                                    ct2[:, :], ps2[:, :], AF.Tanh,
                                    bias=bc_sb[:, ch:ch + 1],
                                )
                                cts.append(ct2)
                            s["ct_r"] = cts

                        def stage_S(k, r):
                            t0, N = M_TILES[k]
                            s = st[k]
                            last = r == ni
                            for ch in range(KC):
                                ct_in = (s["ct"][:, ch, :N] if r == 0
                                         else s["ct_r"][ch][:, :])
                                b = wk.tile([128, N], F16, tag="b1")
                                nc.vector.tensor_mul(b[:, :], s["i"][:, ch, :N], ct_in)
                                dst = Cf if last else C8
                                nc.vector.tensor_tensor_scan(
                                    dst[:, ch, t0 + 1:t0 + N + 1],
                                    s["f"][:, ch, :N],
                                    b[:, :],
                                    Cf[:, ch, t0:t0 + 1],
                                    ALU.mult,
                                    ALU.add,
                                )
                                if last:
                                    nc.vector.tensor_copy(
                                        C8[:, ch, t0 + 1:t0 + N + 1],
                                        Cf[:, ch, t0 + 1:t0 + N + 1],
                                    )

                        def stage_H(k, _r):
                            t0, N = M_TILES[k]
                            s = st[k]
                            for ch in range(KC):
                                tch = wk.tile([128, N], F16, tag="tch")
                                nc.scalar.activation(
                                    tch[:, :], Cf[:, ch, t0 + 1:t0 + N + 1], AF.Tanh
                                )
                                if light:
                                    nc.vector.tensor_mul(
                                        H8[:, ch, t0 + 1:t0 + N + 1],
                                        s["o"][:, ch, :N], tch[:, :],
                                    )
                                else:
                                    nc.vector.tensor_mul(
                                        Hf[:, ch, t0 + 1:t0 + N + 1],
                                        s["o"][:, ch, :N], tch[:, :],
                                    )

                        return _SweepObj(stage_G, stage_R, stage_S,
                                         lambda k: stage_H(k, 0), ni)

                    sweepsA = [make_sweep_f8(first=True, light=True, ni=1)]
                    for j in range(1, k_f8):
                        sweepsA.append(make_sweep_f8(
                            first=False, light=(j < k_f8 - 1), ni=n_inner))
                    _emit_pipeline(sweepsA)

                # ============ PHASE B: f16 polish sweeps ============
                with tc.tile_pool(name="phB", bufs=1) as pB:
                    wh_sb = pB.tile([128, KC, 4 * D], F16, tag="wh16")
                    wc_sb = pB.tile([128, KC, D], F16, tag="wc16")
                    nc.sync.dma_start(
                        wh_sb[:, :, :],
                        wh16_d[:, :].rearrange("(c p) m -> p c m", p=128),
                    )
                    nc.sync.dma_start(
                        wc_sb[:, :, :],
                        wc16_d[:, :].rearrange("(c p) m -> p c m", p=128),
                    )

                    def mm16(ps, w_sb, src, col0, t0, N, start, stop):
                        for kc in range(KC):
                            nc.tensor.matmul(
                                ps[:, :],
                                w_sb[:, kc, col0:col0 + 128],
                                src[:, kc, t0:t0 + N],
                                start=start and (kc == 0),
                                stop=stop and (kc == KC - 1),
                            )

                    def make_sweep_f16(ni, emit_out=False):
                        st = {}

                        def stage_G(k):
                            t0, N = M_TILES[k]
                            s = st[k] = {}
                            s["f"] = gpool.tile([128, KC, NMAX], F16, tag="f", name="fga")
                            s["i"] = gpool.tile([128, KC, NMAX], F16, tag="i", name="iga")
                            s["o"] = gpool.tile([128, KC, NMAX], F16, tag="o", name="oga")
                            s["ct"] = gpool.tile([128, KC, NMAX], F16, tag="ct", name="ctga")
                            s["zA"] = gpool.tile([128, KC, NMAX], F16,
                                                 tag="zA16", bufs=3, name="zA16")
                            pf_t, pi_t, po_t = prefetch_pre(t0, N)
                            # ctilde r=0
                            for ch in range(KC):
                                psA = psum.tile([128, N], F32, tag="ps")
                                mm16(psA, wh_sb, Hf, 3 * D + ch * 128, t0, N,
                                     True, True)
                                nc.vector.tensor_copy(s["zA"][:, ch, :N], psA[:, :])
                                ps0 = psum.tile([128, N], F32, tag="ps")
                                mm16(ps0, wc_sb, Cf, ch * 128, t0, N, True, False)
                                nc.tensor.matmul(
                                    ps0[:, :], id_sb[:, :], s["zA"][:, ch, :N],
                                    start=False, stop=True,
                                )
                                nc.scalar.activation(
                                    s["ct"][:, ch, :N], ps0[:, :], AF.Tanh,
                                    bias=bc_sb[:, ch:ch + 1],
                                )
                            for garr, pre_tiles, bias_sb, col0 in (
                                (s["f"], pf_t, bf_sb, 0),
                                (s["i"], pi_t, bi_sb, D),
                                (s["o"], po_t, bo_sb, 2 * D),
                            ):
                                for ch in range(KC):
                                    ps = psum.tile([128, N], F32, tag="ps")
                                    mm16(ps, wh_sb, Hf, col0 + ch * 128, t0, N,
                                         True, False)
                                    nc.tensor.matmul(
                                        ps[:, :], id_sb[:, :], pre_tiles[ch][:, :],
                                        start=False, stop=True,
                                    )
                                    nc.scalar.activation(
                                        garr[:, ch, :N], ps[:, :], AF.Sigmoid,
                                        bias=bias_sb[:, ch:ch + 1],
                                    )

                        def stage_R(k, r):
                            t0, N = M_TILES[k]
                            s = st[k]
                            cts = []
                            for ch in range(KC):
                                ps2 = psum.tile([128, N], F32, tag="ps")
                                mm16(ps2, wc_sb, Cf, ch * 128, t0, N, True, False)
                                nc.tensor.matmul(
                                    ps2[:, :], id_sb[:, :], s["zA"][:, ch, :N],
                                    start=False, stop=True,
                                )
                                ct2 = wk.tile([128, N], F16, tag="ct2", bufs=4)
                                nc.scalar.activation(
                                    ct2[:, :], ps2[:, :], AF.Tanh,
                                    bias=bc_sb[:, ch:ch + 1],
                                )
                                cts.append(ct2)
                            s["ct_r"] = cts

                        def stage_S(k, r):
                            t0, N = M_TILES[k]
                            s = st[k]
                            for ch in range(KC):
                                ct_in = (s["ct"][:, ch, :N] if r == 0
                                         else s["ct_r"][ch][:, :])
                                b = wk.tile([128, N], F16, tag="b1")
                                nc.vector.tensor_mul(b[:, :], s["i"][:, ch, :N], ct_in)
                                nc.vector.tensor_tensor_scan(
                                    Cf[:, ch, t0 + 1:t0 + N + 1],
                                    s["f"][:, ch, :N],
                                    b[:, :],
                                    Cf[:, ch, t0:t0 + 1],
                                    ALU.mult,
                                    ALU.add,
                                )

                        def stage_H(k, _r):
                            t0, N = M_TILES[k]
                            s = st[k]
                            for ch in range(KC):
                                tch = wk.tile([128, N], F16, tag="tch")
                                nc.scalar.activation(
                                    tch[:, :], Cf[:, ch, t0 + 1:t0 + N + 1], AF.Tanh
                                )
                                nc.vector.tensor_mul(
                                    Hf[:, ch, t0 + 1:t0 + N + 1],
                                    s["o"][:, ch, :N], tch[:, :],
                                )
                            if emit_out:
                                nc.sync.dma_start(
                                    h_out[:, :, t0 + 1:t0 + N + 1],
                                    Hf[:, :, t0 + 1:t0 + N + 1],
                                )

                        return _SweepObj(stage_G, stage_R, stage_S,
                                         lambda k: stage_H(k, 0), ni)

                    # boundary column of the output never changes after init
                    nc.sync.dma_start(h_out[:, :, 0:1], Hf[:, :, 0:1])
                    _emit_pipeline([make_sweep_f16(n_inner, emit_out=(j == k_f16 - 1))
                                    for j in range(k_f16)])

    nc.compile()
    return nc


# ------------------------- host side -------------------------

def _prep_core_inputs(inputs):
    """Build the 8 per-core input maps from the full problem inputs."""
    x = np.asarray(inputs["target_seq"], np.float32)
    W_f = np.asarray(inputs["W_f"], np.float32)
    W_i = np.asarray(inputs["W_i"], np.float32)
    W_C = np.asarray(inputs["W_C"], np.float32)
    W_o = np.asarray(inputs["W_o"], np.float32)

    wh16 = np.concatenate(
        [W_f[:, :D].T, W_i[:, :D].T, W_o[:, :D].T, W_C[:, :D].T], axis=1
    ).astype(np.float16)                      # [D, 4D], cols = [f|i|o|C]
    wc16 = np.ascontiguousarray(W_C[:, D:].T).astype(np.float16)   # [D, D]
    wh8 = wh16.astype(ml_dtypes.float8_e4m3)
    wc8 = wc16.astype(ml_dtypes.float8_e4m3)
    wx_t = np.concatenate(
        [W_f[:, D:].T, W_i[:, D:].T, W_o[:, D:].T], axis=1
    ).astype(np.float16)                      # [D, 3D]

    def vec_pc(v):  # [D] -> [128, 8] with d = ch*128 + p
        return np.ascontiguousarray(np.asarray(v, np.float32).reshape(KC, 128).T)

    bias_f = vec_pc(inputs["b_f"])
    bias_i = vec_pc(inputs["b_i"])
    bias_o = vec_pc(inputs["b_o"])
    bias_c = vec_pc(inputs["b_C"])
    ident = np.eye(128, dtype=np.float16)

    h0 = np.asarray(inputs["encoder_h"], np.float32)
    c0 = np.asarray(inputs["encoder_c"], np.float32)

    in_maps = []
    for core in range(N_CORES):
        if core == 0:
            rows = slice(0, L)
            hbv = vec_pc(h0).astype(np.float16)[:, :, None]
            cbv = vec_pc(c0).astype(np.float16)[:, :, None]
        else:
            rows = slice(1024 * core - DELTA, 1024 * core + 1024)
            hbv = np.zeros((128, KC, 1), np.float16)
            cbv = np.zeros((128, KC, 1), np.float16)
        x_chunk_t = np.ascontiguousarray(x[rows].T).astype(np.float16)  # [D, L]
        in_maps.append({
            "wh8": wh8, "wc8": wc8, "wh16": wh16, "wc16": wc16, "wx_t": wx_t,
            "x_t": x_chunk_t,
            "bias_f": bias_f, "bias_i": bias_i, "bias_o": bias_o, "bias_c": bias_c,
            "hb": hbv, "cb": cbv, "ident": ident,
        })
    return in_maps


def _gather_output(results):
    """Assemble [T, D] fp32 from per-core h_out [128, 8, L+1] fp16."""
    out = np.empty((T, D), np.float32)
    for core in range(N_CORES):
        h = np.asarray(results[core]["h_out"]).reshape(128, KC, L + 1)
        # col j = h at local time j-1 ; d = ch*128 + p
        chunk = np.transpose(h, (2, 1, 0)).reshape(L + 1, D).astype(np.float32)
        if core == 0:
            out[0:1024] = chunk[1:1025]
        else:
            out[1024 * core:1024 * (core + 1)] = chunk[DELTA + 1:L + 1]
    return out


_NC_CACHE = {}


def _get_nc(k_f8=K_F8, k_f16=K_F16, n_inner=N_INNER):
    key = (k_f8, k_f16, n_inner)
    if key not in _NC_CACHE:
        _NC_CACHE[key] = build_nc(k_f8, k_f16, n_inner)
    return _NC_CACHE[key]


def kernel(**inputs) -> np.ndarray:
    nc = _get_nc()
    in_maps = _prep_core_inputs(inputs)
    res = run_bass_kernel_spmd(nc, in_maps, list(range(N_CORES)))
    return _gather_output(res.results)


if __name__ == "__main__":
    nc = build_nc()
    print("built ok")
